# revision 1
# baseline (speedup 1.0000x reference)
"""DeltaNet model kernel for 8 Trainium2 NeuronCores.

Sharding: data-parallel over batch (2) x tensor-parallel over vocab (4) for
the LM head; each core runs the full 2-layer backbone for its batch element
and computes logits for its 8000-vocab shard.  No inter-core communication.

The delta-rule scan is evaluated in closed "chunked attention" form
(chunk=128): per-chunk inverse of (I + strict_tril(beta * K K^T)) via exact
nilpotent squaring, then all cross-chunk interactions as dense matmuls.

Numerics: float32r (fp32 streamed at bf16 rate, ~12-bit mantissa products,
fp32 accumulate) everywhere except the chunk-inverse iteration (bf16).
All weights are pre-rounded to the f32r grid on the host so DMA-ing them
into f32r tiles is exact.
"""

import sys

for _p in ("/opt/trn_rl_repo",):
    if _p not in sys.path:
        sys.path.insert(0, _p)

import numpy as np

import concourse.bass as bass
import concourse.mybir as mybir
from concourse import bacc
from concourse.bass_utils import run_bass_kernel_spmd
from concourse.tile import TileContext
from concourse.masks import make_identity, make_upper_triangular

P = 128
D = 1024
S = 1024
V = 32000
L = 2
NCH = 8           # token chunks of 128
DSUB = 8          # D / P
VS = V // 4       # vocab shard = 8000
VTS = 63          # padded v-tiles (63*128 = 8064)
VSP = VTS * P

F32 = mybir.dt.float32
F32R = mybir.dt.float32r
BF16 = mybir.dt.bfloat16
I32 = mybir.dt.int32
AF = mybir.ActivationFunctionType
ALU = mybir.AluOpType

EPS_L2 = 1e-6
EPS_RMS = 1e-5
EPS_LN = 1e-5


def ts(i, n):
    return slice(i * n, (i + 1) * n)


def build_program():
    nc = bacc.Bacc("TRN2", target_bir_lowering=False, debug=False, num_devices=8)

    tok_d = nc.dram_tensor("tokens", (P, NCH), I32, kind="ExternalInput").ap()
    emb_d = nc.dram_tensor("emb", (V, D), F32R, kind="ExternalInput").ap()
    wq_d = nc.dram_tensor("wq", (L, P, DSUB, D), F32R, kind="ExternalInput").ap()
    wk_d = nc.dram_tensor("wk", (L, P, DSUB, D), F32R, kind="ExternalInput").ap()
    wv_d = nc.dram_tensor("wv", (L, P, DSUB, D), F32R, kind="ExternalInput").ap()
    wb_d = nc.dram_tensor("wb", (L, P, DSUB, 2), F32R, kind="ExternalInput").ap()
    wo_d = nc.dram_tensor("wo", (L, P, DSUB, D), F32R, kind="ExternalInput").ap()
    lng_d = nc.dram_tensor("lng", (P, DSUB), F32, kind="ExternalInput").ap()
    lnb_d = nc.dram_tensor("lnb", (P, DSUB), F32, kind="ExternalInput").ap()
    hw_d = nc.dram_tensor("hw", (VTS, P, DSUB, P), F32R, kind="ExternalInput").ap()
    out_d = nc.dram_tensor("logits_t", (VSP, S), F32, kind="ExternalOutput").ap()

    with TileContext(nc) as tc:
        _build(nc, tc, tok_d, emb_d, wq_d, wk_d, wv_d, wb_d, wo_d,
               lng_d, lnb_d, hw_d, out_d)
    nc.compile()
    return nc


def _build(nc, tc, tok_d, emb_d, wq_d, wk_d, wv_d, wb_d, wo_d,
           lng_d, lnb_d, hw_d, out_d):
    from contextlib import ExitStack
    ctx = ExitStack()
    pool = ctx.enter_context(tc.tile_pool(name="main", bufs=1))
    ring = ctx.enter_context(tc.tile_pool(name="ring", bufs=2))
    scr = ctx.enter_context(tc.tile_pool(name="scr", bufs=2))
    wpool = ctx.enter_context(tc.tile_pool(name="w", bufs=2))
    hppool = ctx.enter_context(tc.tile_pool(name="hp", bufs=8))
    xpool = ctx.enter_context(tc.tile_pool(name="xs", bufs=7))
    sm2 = ctx.enter_context(tc.tile_pool(name="sm2", bufs=2))
    sm4 = ctx.enter_context(tc.tile_pool(name="sm4", bufs=4))
    sm8 = ctx.enter_context(tc.tile_pool(name="sm8", bufs=8))
    rows = ctx.enter_context(tc.tile_pool(name="rows", bufs=5))
    outp = ctx.enter_context(tc.tile_pool(name="outp", bufs=2))
    hwp = ctx.enter_context(tc.tile_pool(name="hwp", bufs=3))
    dram = ctx.enter_context(tc.tile_pool(name="dram", bufs=1, space="DRAM"))
    pa = ctx.enter_context(tc.tile_pool(name="pa", bufs=4, space="PSUM"))
    pb = ctx.enter_context(tc.tile_pool(name="pb", bufs=4, space="PSUM"))

    # ---- constants ----
    ident_f = pool.tile([P, P], F32, tag="identf")
    make_identity(nc, ident_f[:])
    ident_r = pool.tile([P, P], F32R, tag="identr")
    nc.vector.tensor_copy(ident_r[:], ident_f[:])
    mask_ui = pool.tile([P, P], F32, tag="mui")      # 1 where i <= t (upper incl)
    make_upper_triangular(nc, mask_ui[:], val=1.0, diag=True)
    mask_su = pool.tile([P, P], F32, tag="msu")      # 1 where i < t (strict upper)
    make_upper_triangular(nc, mask_su[:], val=1.0, diag=False)
    ones_f = pool.tile([P, 1], F32, tag="onesf")
    nc.gpsimd.memset(ones_f[:], 1.0)
    ones_r = pool.tile([P, 1], F32R, tag="onesr")    # ones column (f32r)
    nc.vector.tensor_copy(ones_r[:], ones_f[:])
    ones_row = pool.tile([1, P], F32, tag="onesrow")  # ones row for bcast
    nc.gpsimd.memset(ones_row[:], 1.0)
    eps6_t = pool.tile([1, 1], F32, tag="eps6")   # 1e-6 (l2norm)
    nc.gpsimd.memset(eps6_t[:], EPS_L2)
    eps5_t = pool.tile([1, 1], F32, tag="eps5")   # 1e-5 (rms / ln)
    nc.gpsimd.memset(eps5_t[:], EPS_RMS)
    lng_sb = pool.tile([P, DSUB], F32, tag="lng")
    nc.sync.dma_start(lng_sb[:], lng_d[:])
    lnb_sb = pool.tile([P, DSUB], F32, tag="lnb")
    nc.sync.dma_start(lnb_sb[:], lnb_d[:])

    # ---- residual stream (feature-major): xfm[p, do, s] = x[s, do*128+p] ----
    xfm = pool.tile([P, DSUB, S], F32R, tag="xfm")

    # ---- embedding gather + transpose to feature-major ----
    tok_sb = pool.tile([P, NCH], I32, tag="tok")
    nc.sync.dma_start(tok_sb[:], tok_d[:])
    for st in range(NCH):
        xg = ring.tile([P, D], F32R, tag="vc")
        nc.gpsimd.indirect_dma_start(
            out=xg[:], out_offset=None, in_=emb_d[:],
            in_offset=bass.IndirectOffsetOnAxis(ap=tok_sb[:, st:st + 1], axis=0))
        for do in range(DSUB):
            pt = pb.tile([P, 256], F32R, tag="pb")
            nc.tensor.transpose(pt[:, :P], xg[:, ts(do, P)], ident_r[:])
            nc.vector.tensor_copy(xfm[:, do, ts(st, P)], pt[:, :P])

    kfm = pool.tile([P, DSUB, S], F32R, tag="kfm")
    u_tm = pool.tile([P, NCH, D], F32R, tag="u")
    beta_tm = pool.tile([P, NCH], F32, tag="btm")
    beta_fm = pool.tile([1, S], F32, tag="bfm")

    for l in range(L):
        # ==== k projection (feature-major) + silu ====
        for dkt in range(DSUB):  # 128-wide chunks of the dk output dim
            wt = wpool.tile([P, DSUB, P], F32R, tag="w")
            nc.sync.dma_start(wt[:], wk_d[l, :, :, ts(dkt, P)])
            for sh in range(2):       # 512-wide s halves
                ps = pa.tile([P, 512], F32, tag="pa")
                for ko in range(DSUB):
                    nc.tensor.matmul(ps[:], wt[:, ko, :],
                                     xfm[:, ko, ts(sh, 512)],
                                     start=(ko == 0), stop=(ko == DSUB - 1))
                sc = scr.tile([P, 512], F32, tag="scr")
                nc.scalar.activation(sc[:], ps[:], AF.Sigmoid)
                nc.vector.tensor_tensor(kfm[:, dkt, ts(sh, 512)], ps[:], sc[:],
                                        ALU.mult)
        # l2-norm of k rows: sumsq over dk (partition dim) via ones-matmul
        ssk_ps = [pa.tile([P, 512], F32, tag="pa", name=f"ssk{l}_{i}") for i in range(2)]
        for dkt in range(DSUB):
            for sh in range(2):
                sq = scr.tile([P, 512], F32R, tag="scr")
                nc.vector.tensor_tensor(sq[:], kfm[:, dkt, ts(sh, 512)],
                                        kfm[:, dkt, ts(sh, 512)], ALU.mult)
                nc.tensor.matmul(ssk_ps[sh][:1, :], ones_r[:], sq[:],
                                 start=(dkt == 0), stop=(dkt == DSUB - 1))
        rk_row = rows.tile([1, S], F32, tag="rkrow", bufs=1)
        for sh in range(2):
            s_ = rows.tile([1, 512], F32, tag="srow")
            nc.scalar.activation(s_[:], ssk_ps[sh][:1, :], AF.Sqrt, bias=eps6_t[:])
            nc.vector.reciprocal(rk_row[:, ts(sh, 512)], s_[:])
        for sh in range(2):
            psb = pa.tile([P, 512], F32, tag="pa")
            nc.tensor.matmul(psb[:], ones_row[:], rk_row[:, ts(sh, 512)],
                             start=True, stop=True)
            rk_bc = ring.tile([P, 512], F32, tag="bc")
            nc.vector.tensor_copy(rk_bc[:], psb[:])
            for dkt in range(DSUB):
                nc.vector.tensor_tensor(kfm[:, dkt, ts(sh, 512)],
                                        kfm[:, dkt, ts(sh, 512)], rk_bc[:],
                                        ALU.mult)

        # ==== beta (token-major and feature-major) ====
        wbt = pool.tile([P, DSUB, 2], F32R, tag="wb")
        nc.sync.dma_start(wbt[:], wb_d[l])
        for st in range(NCH):
            psb = pb.tile([P, 256], F32, tag="pb")
            for ko in range(DSUB):
                nc.tensor.matmul(psb[:, :2], xfm[:, ko, ts(st, P)], wbt[:, ko, :],
                                 start=(ko == 0), stop=(ko == DSUB - 1))
            nc.scalar.activation(beta_tm[:, st:st + 1], psb[:, :1], AF.Sigmoid)
        for sh in range(2):
            psb = pa.tile([P, 512], F32, tag="pa")
            for ko in range(DSUB):
                nc.tensor.matmul(psb[:2, :], wbt[:, ko, :], xfm[:, ko, ts(sh, 512)],
                                 start=(ko == 0), stop=(ko == DSUB - 1))
            nc.scalar.activation(beta_fm[:, ts(sh, 512)], psb[:1, :], AF.Sigmoid)

        # ==== v = silu(x Wv), token-major, parked in DRAM scratch ====
        v_dram = dram.tile([NCH, P, D], F32R, tag="vdram", name=f"vdram{l}")
        for wc2 in range(4):
            wt = wpool.tile([P, DSUB, 256], F32R, tag="wv", bufs=1,
                            name=f"wv{l}_{wc2}")
            nc.sync.dma_start(wt[:], wv_d[l, :, :, ts(wc2, 256)])
            for st in range(NCH):
                ps = pb.tile([P, 256], F32, tag="pb")
                for ko in range(DSUB):
                    nc.tensor.matmul(ps[:], xfm[:, ko, ts(st, P)], wt[:, ko, :],
                                     start=(ko == 0), stop=(ko == DSUB - 1))
                sc = scr.tile([P, 512], F32, tag="scr")
                nc.scalar.activation(sc[:, :256], ps[:], AF.Sigmoid)
                vstg = ring.tile([P, 256], F32R, tag="vstg")
                nc.vector.tensor_tensor(vstg[:], ps[:], sc[:, :256], ALU.mult)
                nc.sync.dma_start(v_dram[st, :, ts(wc2, 256)], vstg[:])

        # ==== chunk inverses: P_c = diag(beta) T_c^T, T = (I+A)^-1 ====
        Ptiles = []
        for c in range(NCH):
            jps = pb.tile([P, 256], F32, tag="pb")
            for ko in range(DSUB):
                nc.tensor.matmul(jps[:, :P], kfm[:, ko, ts(c, P)],
                                 kfm[:, ko, ts(c, P)],
                                 start=(ko == 0), stop=(ko == DSUB - 1))
            jcc = sm2.tile([P, P], F32, tag="jcc")
            nc.vector.tensor_copy(jcc[:], jps[:, :P])
            # N = strict_tril(beta_row * J);  N^T = strict_triu(beta_col * J)
            tmp = scr.tile([P, 512], F32, tag="scr")
            nc.vector.tensor_scalar_mul(tmp[:, :P], jcc[:], beta_tm[:, c:c + 1])
            tmp2 = scr.tile([P, 512], F32, tag="scr")
            nc.vector.tensor_tensor(tmp2[:, :P], tmp[:, :P], mask_ui[:], ALU.mult)
            n_bf = xpool.tile([P, P], BF16, tag="xs")
            nc.vector.tensor_tensor(n_bf[:], tmp[:, :P], tmp2[:, :P],
                                    ALU.subtract)
            bps = pb.tile([P, 256], F32, tag="pb")
            nc.tensor.matmul(bps[:, :P], ones_row[:], beta_fm[:, ts(c, P)],
                             start=True, stop=True)
            mb = sm2.tile([P, P], F32, tag="mbeta")
            nc.vector.tensor_tensor(mb[:], bps[:, :P], mask_su[:], ALU.mult)
            nt_bf = sm2.tile([P, P], BF16, tag="nt")
            nc.vector.tensor_tensor(nt_bf[:], mb[:], jcc[:], ALU.mult)
            # squarings: X_k = N^(2^k), Xt_k = X_k^T; matmul(lhsT,rhs)=lhsT^T@rhs
            xs = [n_bf]
            xt_prev = nt_bf
            for kk in range(6):
                psx = pb.tile([P, 256], F32, tag="pb")
                nc.tensor.matmul(psx[:, :P], xt_prev[:], xs[-1][:],
                                 start=True, stop=True)
                x_new = xpool.tile([P, P], BF16, tag="xs")
                nc.vector.tensor_copy(x_new[:], psx[:, :P])
                if kk < 5:
                    psxt = pb.tile([P, 256], F32, tag="pb")
                    nc.tensor.matmul(psxt[:, :P], xs[-1][:], xt_prev[:],
                                     start=True, stop=True)
                    xt_new = sm2.tile([P, P], BF16, tag="xt")
                    nc.vector.tensor_copy(xt_new[:], psxt[:, :P])
                    xt_prev = xt_new
                xs.append(x_new)
            # chain: M = I + Y^64; M += Y^(2^k) M (k=5..1); G = M - Y M  (Y=N^T)
            mcur = sm2.tile([P, P], F32, tag="mcur")
            nc.vector.tensor_tensor(mcur[:], ident_f[:], xs[6][:], ALU.add)
            mb16 = sm2.tile([P, P], BF16, tag="mb16")
            nc.vector.tensor_copy(mb16[:], mcur[:])
            for kk in range(5, 0, -1):
                psm = pb.tile([P, 256], F32, tag="pb")
                nc.tensor.matmul(psm[:, :P], xs[kk][:], mb16[:],
                                 start=True, stop=True)
                mnew = sm2.tile([P, P], F32, tag="mcur")
                nc.vector.tensor_tensor(mnew[:], mcur[:], psm[:, :P], ALU.add)
                mcur = mnew
                mb16 = sm2.tile([P, P], BF16, tag="mb16")
                nc.vector.tensor_copy(mb16[:], mcur[:])
            psm = pb.tile([P, 256], F32, tag="pb")
            nc.tensor.matmul(psm[:, :P], xs[0][:], mb16[:], start=True, stop=True)
            gt = sm2.tile([P, P], F32, tag="gt")
            nc.vector.tensor_tensor(gt[:], mcur[:], psm[:, :P], ALU.subtract)
            p_c = sm8.tile([P, P], F32R, tag="pc")
            nc.vector.tensor_scalar_mul(p_c[:], gt[:], beta_tm[:, c:c + 1])
            Ptiles.append(p_c)

        # ==== scan ====
        for cp in range(4):
            c0, c1 = 2 * cp, 2 * cp + 1
            # --- q chunk (256 tokens), silu, feature-major, unnormalized ---
            qfm = ring.tile([P, DSUB, 256], F32R, tag="qfm", bufs=1)
            for dqt in range(DSUB):
                wt = wpool.tile([P, DSUB, P], F32R, tag="w")
                nc.sync.dma_start(wt[:], wq_d[l, :, :, ts(dqt, P)])
                ps = pb.tile([P, 256], F32, tag="pb")
                for ko in range(DSUB):
                    nc.tensor.matmul(ps[:], wt[:, ko, :],
                                     xfm[:, ko, ts(cp, 256)],
                                     start=(ko == 0), stop=(ko == DSUB - 1))
                sc = scr.tile([P, 512], F32, tag="scr")
                nc.scalar.activation(sc[:, :256], ps[:], AF.Sigmoid)
                nc.vector.tensor_tensor(qfm[:, dqt, :], ps[:], sc[:, :256],
                                        ALU.mult)
            # rq for these 256 tokens
            sq_ps = pa.tile([P, 512], F32, tag="pa")
            for dqt in range(DSUB):
                sq = scr.tile([P, 512], F32R, tag="scr")
                nc.vector.tensor_tensor(sq[:, :256], qfm[:, dqt, :],
                                        qfm[:, dqt, :], ALU.mult)
                nc.tensor.matmul(sq_ps[:1, :256], ones_r[:], sq[:, :256],
                                 start=(dqt == 0), stop=(dqt == DSUB - 1))
            s_ = rows.tile([1, 512], F32, tag="srow")
            nc.scalar.activation(s_[:, :256], sq_ps[:1, :256], AF.Sqrt,
                                 bias=eps6_t[:])
            rq_row = rows.tile([1, 512], F32, tag="srow")
            nc.vector.reciprocal(rq_row[:, :256], s_[:, :256])

            for c in (c0, c1):
                # --- v rows for this chunk (from DRAM scratch) ---
                v_c = ring.tile([P, D], F32R, tag="vc")
                nc.sync.dma_start(v_c[:], v_dram[c])
                # --- J pair tiles for j < c (kept across both halves) ---
                jsbs = []
                for jp in range((c + 1) // 2):
                    jps = pb.tile([P, 256], F32, tag="pb")
                    for ko in range(DSUB):
                        nc.tensor.matmul(jps[:], kfm[:, ko, ts(c, P)],
                                         kfm[:, ko, ts(jp, 256)],
                                         start=(ko == 0), stop=(ko == DSUB - 1))
                    jsb = sm4.tile([P, 256], F32R, tag="jsb")
                    nc.vector.tensor_copy(jsb[:], jps[:])
                    jsbs.append(jsb)
                # --- U_c = (T B) V_c - sum_j G_cj U_j ---
                js = list(range(c))
                for half in range(2):
                    gnegs = []
                    for j in js:
                        gps = pb.tile([P, 256], F32, tag="pb")
                        nc.tensor.matmul(gps[:, :P], jsbs[j // 2][:, ts(j % 2, P)],
                                         Ptiles[c][:], start=True, stop=True)
                        gneg = sm8.tile([P, P], F32R, tag="gneg", bufs=3)
                        nc.vector.tensor_scalar_mul(gneg[:], gps[:, :P], -1.0)
                        gnegs.append(gneg)
                    psu = pa.tile([P, 512], F32, tag="pa")
                    nc.tensor.matmul(psu[:], Ptiles[c][:], v_c[:, ts(half, 512)],
                                     start=True, stop=(len(js) == 0))
                    for gi, j in enumerate(js):
                        nc.tensor.matmul(psu[:], gnegs[gi][:],
                                         u_tm[:, j, ts(half, 512)],
                                         start=False, stop=(gi == len(js) - 1))
                    nc.vector.tensor_copy(u_tm[:, c, ts(half, 512)], psu[:])

            # --- H^T pair tiles for this cp ---
            hps = []
            for j in range(c1 + 1):
                php = pb.tile([P, 256], F32, tag="pb")
                for ko in range(DSUB):
                    nc.tensor.matmul(php[:], kfm[:, ko, ts(j, P)], qfm[:, ko, :],
                                     start=(ko == 0), stop=(ko == DSUB - 1))
                hp = hppool.tile([P, 256], F32R, tag="hp")
                if j == c0:
                    nc.vector.tensor_tensor(hp[:, :P], php[:, :P], mask_ui[:],
                                            ALU.mult)
                    nc.vector.tensor_copy(hp[:, P:], php[:, P:])
                elif j == c1:
                    nc.vector.tensor_tensor(hp[:, P:], php[:, P:], mask_ui[:],
                                            ALU.mult)
                else:
                    nc.vector.tensor_copy(hp[:], php[:])
                hps.append(hp)
            # --- O feature-major, accumulate over j per e-tile ---
            on_c = ring.tile([P, DSUB, 256], F32R, tag="on", bufs=1)
            sso_ps = pa.tile([P, 512], F32, tag="pa")
            for wave in range(2):
                opss = []
                for ei in range(4):
                    et = wave * 4 + ei
                    pso = pb.tile([P, 256], F32, tag="pb")
                    for j in range(c1 + 1):
                        if j == c1:
                            nc.tensor.matmul(pso[:, P:], u_tm[:, j, ts(et, P)],
                                             hps[j][:, P:], start=False, stop=True)
                        else:
                            nc.tensor.matmul(pso[:], u_tm[:, j, ts(et, P)],
                                             hps[j][:], start=(j == 0), stop=False)
                    opss.append((et, pso))
                for et, pso in opss:
                    nc.vector.tensor_copy(on_c[:, et, :], pso[:])
                    sq = scr.tile([P, 512], F32R, tag="scr")
                    nc.vector.tensor_tensor(sq[:, :256], on_c[:, et, :],
                                            on_c[:, et, :], ALU.mult)
                    nc.tensor.matmul(sso_ps[:1, :256], ones_r[:], sq[:, :256],
                                     start=(et == 0), stop=(et == DSUB - 1))
            # combined scale row: a = rq / sqrt(rq^2 * sso / D + eps_rms)
            rq2 = rows.tile([1, 512], F32, tag="srow")
            nc.vector.tensor_tensor(rq2[:, :256], rq_row[:, :256],
                                    rq_row[:, :256], ALU.mult)
            nc.vector.tensor_scalar_mul(rq2[:, :256], rq2[:, :256], 1.0 / D)
            ssos = rows.tile([1, 512], F32, tag="srow")
            nc.vector.tensor_tensor(ssos[:, :256], sso_ps[:1, :256], rq2[:, :256],
                                    ALU.mult)
            nc.scalar.activation(ssos[:, :256], ssos[:, :256], AF.Sqrt,
                                 bias=eps5_t[:])
            row_a = rows.tile([1, 512], F32, tag="srow")
            nc.vector.reciprocal(row_a[:, :256], ssos[:, :256])
            nc.vector.tensor_tensor(row_a[:, :256], row_a[:, :256],
                                    rq_row[:, :256], ALU.mult)
            psb = pb.tile([P, 256], F32, tag="pb")
            nc.tensor.matmul(psb[:], ones_row[:], row_a[:, :256],
                             start=True, stop=True)
            a_bc = sm2.tile([P, 256], F32, tag="abc")
            nc.vector.tensor_copy(a_bc[:], psb[:])
            for et in range(DSUB):
                nc.vector.tensor_tensor(on_c[:, et, :], on_c[:, et, :], a_bc[:],
                                        ALU.mult)

            # --- x_next columns for this cp ---
            for do in range(DSUB):
                wt = wpool.tile([P, DSUB, P], F32R, tag="w")
                nc.sync.dma_start(wt[:], wo_d[l, :, :, ts(do, P)])
                psx = pb.tile([P, 256], F32, tag="pb")
                for ko in range(DSUB):
                    nc.tensor.matmul(psx[:], wt[:, ko, :],
                                     on_c[:, ko, :],
                                     start=(ko == 0), stop=(ko == DSUB - 1))
                nc.vector.tensor_copy(xfm[:, do, ts(cp, 256)], psx[:])

    # ==== final layernorm (feature-major) ====
    sum_ps = [pa.tile([P, 512], F32, tag="pa", name=f"lnsum{i}") for i in range(2)]
    ssq_ps = [pa.tile([P, 512], F32, tag="pa", name=f"lnssq{i}") for i in range(2)]
    for do in range(DSUB):
        for sh in range(2):
            nc.tensor.matmul(sum_ps[sh][:1, :], ones_r[:], xfm[:, do, ts(sh, 512)],
                             start=(do == 0), stop=(do == DSUB - 1))
            sq = scr.tile([P, 512], F32R, tag="scr")
            nc.vector.tensor_tensor(sq[:], xfm[:, do, ts(sh, 512)],
                                    xfm[:, do, ts(sh, 512)], ALU.mult)
            nc.tensor.matmul(ssq_ps[sh][:1, :], ones_r[:], sq[:],
                             start=(do == 0), stop=(do == DSUB - 1))
    # per-half: row stats -> broadcast -> apply (xn in place on xfm)
    for sh in range(2):
        mu = rows.tile([1, 512], F32, tag="srow")
        nc.vector.tensor_scalar_mul(mu[:], sum_ps[sh][:1, :], 1.0 / D)
        m2_ = rows.tile([1, 512], F32, tag="srow")
        nc.vector.tensor_scalar_mul(m2_[:], ssq_ps[sh][:1, :], 1.0 / D)
        mu2 = rows.tile([1, 512], F32, tag="srow")
        nc.vector.tensor_tensor(mu2[:], mu[:], mu[:], ALU.mult)
        nc.vector.tensor_tensor(m2_[:], m2_[:], mu2[:], ALU.subtract)
        nc.scalar.activation(mu2[:], m2_[:], AF.Sqrt, bias=eps5_t[:])
        row_a = rows.tile([1, 512], F32, tag="srow")
        nc.vector.reciprocal(row_a[:], mu2[:])
        nc.vector.tensor_scalar_mul(mu[:], mu[:], -1.0)
        row_b = rows.tile([1, 512], F32, tag="srow")
        nc.vector.tensor_tensor(row_b[:], mu[:], row_a[:], ALU.mult)
        psb = pa.tile([P, 512], F32, tag="pa")
        nc.tensor.matmul(psb[:], ones_row[:], row_a[:], start=True, stop=True)
        a_bc = ring.tile([P, 512], F32, tag="bc")
        nc.vector.tensor_copy(a_bc[:], psb[:])
        psb = pa.tile([P, 512], F32, tag="pa")
        nc.tensor.matmul(psb[:], ones_row[:], row_b[:], start=True, stop=True)
        b_bc = ring.tile([P, 512], F32, tag="bc")
        nc.vector.tensor_copy(b_bc[:], psb[:])
        for do in range(DSUB):
            t1 = scr.tile([P, 512], F32, tag="scr")
            nc.vector.tensor_tensor(t1[:], xfm[:, do, ts(sh, 512)], a_bc[:],
                                    ALU.mult)
            nc.vector.tensor_tensor(t1[:], t1[:], b_bc[:], ALU.add)
            nc.vector.tensor_scalar(t1[:], t1[:], lng_sb[:, do:do + 1],
                                    lnb_sb[:, do:do + 1], ALU.mult, ALU.add)
            nc.vector.tensor_copy(xfm[:, do, ts(sh, 512)], t1[:])

    # ==== vocab-shard head: logits_t[vt*128+vv, s] ====
    for vt in range(VTS):
        hwts = []
        for kw in range(2):
            hwt = hwp.tile([P, 4, P], F32R, tag="hw", name=f"hw{vt}_{kw}")
            nc.sync.dma_start(hwt[:], hw_d[vt, :, ts(kw, 4), :])
            hwts.append(hwt)
        for sh in range(2):
            ps = pa.tile([P, 512], F32, tag="pa")
            for ko in range(DSUB):
                nc.tensor.matmul(ps[:], hwts[ko // 4][:, ko % 4, :],
                                 xfm[:, ko, ts(sh, 512)],
                                 start=(ko == 0), stop=(ko == DSUB - 1))
            ot = outp.tile([P, 512], F32, tag="out")
            nc.vector.tensor_copy(ot[:], ps[:])
            nc.sync.dma_start(out_d[ts(vt, P), ts(sh, 512)], ot[:])

    ctx.close()


def _round_f32r(x):
    m, e = np.frexp(x.astype(np.float64))
    return np.ldexp(np.round(m * 4096.0) / 4096.0, e).astype(np.float32)


_CACHE = {}


def _get_program():
    if "nc" not in _CACHE:
        _CACHE["nc"] = build_program()
    return _CACHE["nc"]


def make_in_maps(tokens, emb, Wq, Wk, Wv, Wb, Wo, rms_w, ln_g, ln_b, head_w):
    def arrange_w(w):  # [D, N] -> [128, DSUB, N] with (p, ko) striping of D
        return np.ascontiguousarray(
            _round_f32r(w).reshape(DSUB, P, -1).transpose(1, 0, 2))

    wq_h = np.stack([arrange_w(Wq[l]) for l in range(L)])
    wk_h = np.stack([arrange_w(Wk[l]) for l in range(L)])
    wv_h = np.stack([arrange_w(Wv[l]) for l in range(L)])
    wb_h = np.stack([arrange_w(np.repeat(Wb[l], 2, axis=1)) for l in range(L)])
    wo_h = np.stack([arrange_w(rms_w[l][:, None] * Wo[l]) for l in range(L)])
    emb_h = _round_f32r(emb)
    lng_h = np.ascontiguousarray(ln_g.reshape(DSUB, P).T)
    lnb_h = np.ascontiguousarray(ln_b.reshape(DSUB, P).T)

    in_maps = []
    for core in range(8):
        b, vs = core // 4, core % 4
        hw_pad = np.zeros((D, VSP), np.float32)
        hw_pad[:, :VS] = _round_f32r(head_w[:, ts(vs, VS)])
        hw_h = np.ascontiguousarray(
            hw_pad.reshape(DSUB, P, VTS, P).transpose(2, 1, 0, 3))
        tok_h = np.ascontiguousarray(
            tokens[b].astype(np.int32).reshape(NCH, P).T)
        in_maps.append({
            "tokens": tok_h, "emb": emb_h,
            "wq": wq_h, "wk": wk_h, "wv": wv_h, "wb": wb_h, "wo": wo_h,
            "lng": lng_h, "lnb": lnb_h, "hw": hw_h,
        })
    return in_maps


def assemble_output(results):
    out = np.empty((2, S, V), np.float32)
    for core in range(8):
        b, vs = core // 4, core % 4
        lt = results[core]["logits_t"]          # [VSP, S]
        out[b, :, ts(vs, VS)] = np.ascontiguousarray(lt[:VS]).T
    return out


def kernel(tokens, emb, Wq, Wk, Wv, Wb, Wo, rms_w, ln_g, ln_b, head_w):
    tokens = np.asarray(tokens)
    args = [np.asarray(a, np.float32) for a in
            (emb, Wq, Wk, Wv, Wb, Wo, rms_w, ln_g, ln_b, head_w)]
    nc = _get_program()
    in_maps = make_in_maps(tokens, *args)
    res = run_bass_kernel_spmd(nc, in_maps, core_ids=list(range(8)),
                               trace=bool(_CACHE.get("trace")))
    _CACHE["last_result"] = res
    return assemble_output(res.results)



# revision 13
# speedup vs baseline: 1.5860x; 1.5860x over previous
"""DeltaNet model kernel for 8 Trainium2 NeuronCores.

Sharding: data-parallel over batch (2) x tensor-parallel over vocab (4) for
the LM head; each core runs the full 2-layer backbone for its batch element
and computes logits for its 8000-vocab shard.  No inter-core communication.

The delta-rule scan is evaluated in closed "chunked attention" form
(chunk=128): per-chunk inverse of (I + strict_tril(beta * K K^T)) via exact
nilpotent squaring, then all cross-chunk interactions as dense matmuls.

v2 restructure for PE density:
 - silu fused on the scalar engine (AF.Silu), sigmoid via tanh (same ACT
   table set) -> 4 table loads total
 - norm stats kept token-per-partition ("columns") so reciprocals run wide
 - rms scale commuted past Wo, applied once at end of layer
 - q/o projections hoisted out of the chunk loop (weights DMA'd once)
 - v parked in SBUF (u_tm slots), overwritten by U in place
 - bf16 operands for every small-N matmul (f32r is 4 cyc/row below N=256)
"""

import sys

for _p in ("/opt/trn_rl_repo",):
    if _p not in sys.path:
        sys.path.insert(0, _p)

import ml_dtypes
import numpy as np

import concourse.bass as bass
import concourse.mybir as mybir
from concourse import bacc
from concourse.bass_utils import run_bass_kernel_spmd
from concourse.tile import TileContext
from concourse.masks import make_identity, make_upper_triangular

P = 128
D = 1024
S = 1024
V = 32000
L = 2
NCH = 8           # token chunks of 128
DSUB = 8          # D / P
VS = V // 4       # vocab shard = 8000
VTS = 63          # padded v-tiles (63*128 = 8064)
VSP = VTS * P

F32 = mybir.dt.float32
F32R = mybir.dt.float32r
BF16 = mybir.dt.bfloat16
I32 = mybir.dt.int32
AF = mybir.ActivationFunctionType
ALU = mybir.AluOpType

EPS_L2 = 1e-6
EPS_RMS = 1e-5
EPS_LN = 1e-5


def ts(i, n):
    return slice(i * n, (i + 1) * n)


def build_program():
    nc = bacc.Bacc("TRN2", target_bir_lowering=False, debug=False, num_devices=8)

    tok_d = nc.dram_tensor("tokens", (P, NCH), I32, kind="ExternalInput").ap()
    emb_d = nc.dram_tensor("emb", (V, D), F32R, kind="ExternalInput").ap()
    wq_d = nc.dram_tensor("wq", (L, P, DSUB, D), F32R, kind="ExternalInput").ap()
    wk_d = nc.dram_tensor("wk", (L, P, DSUB, D), F32R, kind="ExternalInput").ap()
    wv_d = nc.dram_tensor("wv", (L, P, DSUB, D), F32R, kind="ExternalInput").ap()
    wb_d = nc.dram_tensor("wb", (L, P, DSUB, 2), F32R, kind="ExternalInput").ap()
    wo_d = nc.dram_tensor("wo", (L, P, DSUB, D), BF16, kind="ExternalInput").ap()
    hw_d = nc.dram_tensor("hw", (VTS, P, DSUB, P), F32R, kind="ExternalInput").ap()
    hb_d = nc.dram_tensor("hb", (P, VTS), F32, kind="ExternalInput").ap()
    out_d = nc.dram_tensor("logits_t", (VSP, S), BF16, kind="ExternalOutput").ap()

    with TileContext(nc) as tc:
        _build(nc, tc, tok_d, emb_d, wq_d, wk_d, wv_d, wb_d, wo_d, hw_d, hb_d,
               out_d)
    nc.compile()
    return nc


def _build(nc, tc, tok_d, emb_d, wq_d, wk_d, wv_d, wb_d, wo_d, hw_d, hb_d,
           out_d):
    from contextlib import ExitStack
    ctx = ExitStack()
    pool = ctx.enter_context(tc.tile_pool(name="main", bufs=1))
    invf = ctx.enter_context(tc.tile_pool(name="invf", bufs=1))
    invb = ctx.enter_context(tc.tile_pool(name="invb", bufs=1))
    jpool = ctx.enter_context(tc.tile_pool(name="jsb", bufs=4))
    gpool = ctx.enter_context(tc.tile_pool(name="gneg", bufs=2))
    hppool = ctx.enter_context(tc.tile_pool(name="hp", bufs=8))
    wko = ctx.enter_context(tc.tile_pool(name="wko", bufs=2))
    wvp = ctx.enter_context(tc.tile_pool(name="wvp", bufs=1))
    scr = ctx.enter_context(tc.tile_pool(name="scr", bufs=2))
    ring = ctx.enter_context(tc.tile_pool(name="ring", bufs=2))
    outp = ctx.enter_context(tc.tile_pool(name="outp", bufs=2))
    hwp = ctx.enter_context(tc.tile_pool(name="hwp", bufs=3))
    smallf = ctx.enter_context(tc.tile_pool(name="smallf", bufs=2))
    rowp = ctx.enter_context(tc.tile_pool(name="rowp", bufs=3))
    pa = ctx.enter_context(tc.tile_pool(name="pa", bufs=4, space="PSUM"))
    pb = ctx.enter_context(tc.tile_pool(name="pb", bufs=4, space="PSUM"))

    # ---- constants ----
    ident_f = pool.tile([P, P], F32, tag="identf")
    make_identity(nc, ident_f[:])
    ident_r = pool.tile([P, P], F32R, tag="identr")
    nc.vector.tensor_copy(ident_r[:], ident_f[:])
    mask_ui = pool.tile([P, P], F32, tag="mui")      # 1 where i <= t (upper incl)
    make_upper_triangular(nc, mask_ui[:], val=1.0, diag=True)
    mask_su = pool.tile([P, P], F32, tag="msu")      # 1 where i < t (strict upper)
    make_upper_triangular(nc, mask_su[:], val=1.0, diag=False)
    mask_sl = pool.tile([P, P], F32, tag="msl")      # 1 where i > t (strict lower)
    nc.gpsimd.memset(mask_sl[:], 1.0)
    nc.vector.tensor_tensor(mask_sl[:], mask_sl[:], mask_ui[:], ALU.subtract)
    ones_f = pool.tile([P, P], F32, tag="onesf")
    nc.gpsimd.memset(ones_f[:], 1.0)
    ones_r = pool.tile([P, 1], F32R, tag="onesr")
    nc.vector.tensor_copy(ones_r[:], ones_f[:, :1])
    ones_sq = pool.tile([P, P], F32R, tag="onessq")
    nc.vector.tensor_copy(ones_sq[:], ones_f[:])
    zeros_sb = pool.tile([P, 512], F32, tag="zeros")
    nc.gpsimd.memset(zeros_sb[:], 0.0)
    ones_row = pool.tile([1, P], F32, tag="onesrow")
    nc.gpsimd.memset(ones_row[:], 1.0)
    hb_sb = pool.tile([P, VTS], F32, tag="hb")
    nc.sync.dma_start(hb_sb[:], hb_d[:])
    eps6_t = pool.tile([P, 1], F32, tag="eps6")
    nc.gpsimd.memset(eps6_t[:], EPS_L2)
    eps5_t = pool.tile([P, 1], F32, tag="eps5")
    nc.gpsimd.memset(eps5_t[:], EPS_RMS)
    half_t = pool.tile([P, 1], F32, tag="half")
    nc.gpsimd.memset(half_t[:], 0.5)
    invd_t = pool.tile([P, 1], F32, tag="invd")
    nc.gpsimd.memset(invd_t[:], 1.0 / D)

    # ---- residual stream (feature-major): xfm[p, do, s] = x[s, do*128+p] ----
    xfm = pool.tile([P, DSUB, S], F32R, tag="xfm")
    kfm = pool.tile([P, DSUB, S], BF16, tag="kfm")
    qfm = pool.tile([P, DSUB, S], BF16, tag="qfm")   # q, then scan output `on`
    u_tm = pool.tile([P, NCH, D], F32R, tag="u")     # v, then U (in place)
    beta_tm = pool.tile([P, NCH], F32, tag="btm")

    # ---- embedding gather (into u_tm scratch) + transpose to feature-major ----
    tok_sb = pool.tile([P, NCH], I32, tag="tok")
    nc.sync.dma_start(tok_sb[:], tok_d[:])
    with nc.named_scope("embed"):
        for st in range(NCH):
            nc.gpsimd.indirect_dma_start(
                out=u_tm[:, st, :], out_offset=None, in_=emb_d[:],
                in_offset=bass.IndirectOffsetOnAxis(ap=tok_sb[:, st:st + 1], axis=0))
            for dp in range(4):
                pt = pb.tile([P, 256], F32R, tag="pb")
                nc.tensor.transpose(pt[:, :P], u_tm[:, st, ts(2 * dp, P)], ident_r[:])
                nc.tensor.transpose(pt[:, P:], u_tm[:, st, ts(2 * dp + 1, P)], ident_r[:])
                nc.any.tensor_copy(xfm[:, 2 * dp, ts(st, P)], pt[:, :P])
                nc.any.tensor_copy(xfm[:, 2 * dp + 1, ts(st, P)], pt[:, P:])

    for l in range(L):
        # ==== projections: k (feature-major, bf16), q (feature-major),
        #      v (token-major, into u_tm), beta ====
        with nc.named_scope(f"L{l}_proj"):
            for dt_ in range(DSUB):
                wt = wko.tile([P, DSUB, P], F32R, tag="w")
                nc.sync.dma_start(wt[:], wk_d[l, :, :, ts(dt_, P)])
                for sh in range(2):
                    ps = pa.tile([P, 512], F32, tag="pa")
                    for ko in range(DSUB):
                        nc.tensor.matmul(ps[:], wt[:, ko, :],
                                         xfm[:, ko, ts(sh, 512)],
                                         start=(ko == 0), stop=(ko == DSUB - 1))
                    nc.scalar.activation(kfm[:, dt_, ts(sh, 512)], ps[:], AF.Silu)
            for dt_ in range(DSUB):
                wt = wko.tile([P, DSUB, P], F32R, tag="w")
                nc.sync.dma_start(wt[:], wq_d[l, :, :, ts(dt_, P)])
                for sh in range(2):
                    ps = pa.tile([P, 512], F32, tag="pa")
                    for ko in range(DSUB):
                        nc.tensor.matmul(ps[:], wt[:, ko, :],
                                         xfm[:, ko, ts(sh, 512)],
                                         start=(ko == 0), stop=(ko == DSUB - 1))
                    nc.scalar.activation(qfm[:, dt_, ts(sh, 512)], ps[:], AF.Silu)
            for wc2 in range(4):
                wv = wvp.tile([P, DSUB, 256], F32R, tag="wv")
                nc.sync.dma_start(wv[:], wv_d[l, :, :, ts(wc2, 256)])
                for st in range(NCH):
                    ps = pb.tile([P, 256], F32, tag="pb")
                    for ko in range(DSUB):
                        nc.tensor.matmul(ps[:], xfm[:, ko, ts(st, P)], wv[:, ko, :],
                                         start=(ko == 0), stop=(ko == DSUB - 1))
                    nc.scalar.activation(u_tm[:, st, ts(wc2, 256)], ps[:], AF.Silu)
            # beta = sigmoid(x Wb) = 0.5*tanh(0.5 z) + 0.5 (tanh shares the
            # silu table set)
            wbt = pool.tile([P, DSUB, 2], F32R, tag="wb")
            nc.sync.dma_start(wbt[:], wb_d[l])
            for st in range(NCH):
                psb = pb.tile([P, 256], F32, tag="pb")
                for ko in range(DSUB):
                    nc.tensor.matmul(psb[:, :2], xfm[:, ko, ts(st, P)], wbt[:, ko, :],
                                     start=(ko == 0), stop=(ko == DSUB - 1))
                nc.scalar.activation(beta_tm[:, st:st + 1], psb[:, :1], AF.Tanh,
                                     scale=half_t[:])
            nc.vector.tensor_scalar(beta_tm[:], beta_tm[:], 0.5, 0.5,
                                    ALU.mult, ALU.add)

        # ==== l2-norm stats for k and q (rows + fast reciprocal) ====
        with nc.named_scope(f"L{l}_norm"):
            ssk_ps = [pa.tile([P, 512], F32, tag="pa", name=f"ssk{l}_{i}")
                      for i in range(2)]
            ssq_ps = [pa.tile([P, 512], F32, tag="pa", name=f"ssq{l}_{i}")
                      for i in range(2)]
            for dt_ in range(DSUB):
                for sh in range(2):
                    sqk = scr.tile([P, 512], F32R, tag="scr")
                    nc.vector.tensor_tensor(sqk[:], kfm[:, dt_, ts(sh, 512)],
                                            kfm[:, dt_, ts(sh, 512)], ALU.mult)
                    nc.tensor.matmul(ssk_ps[sh][:1, :], ones_r[:], sqk[:],
                                     start=(dt_ == 0), stop=(dt_ == DSUB - 1))
                    sqq = scr.tile([P, 512], F32R, tag="scr")
                    nc.vector.tensor_tensor(sqq[:], qfm[:, dt_, ts(sh, 512)],
                                            qfm[:, dt_, ts(sh, 512)], ALU.mult)
                    nc.tensor.matmul(ssq_ps[sh][:1, :], ones_r[:], sqq[:],
                                     start=(dt_ == 0), stop=(dt_ == DSUB - 1))
            ksr = rowp.tile([1, S], F32, tag="rowt")
            qsr = rowp.tile([1, S], F32, tag="rowt")
            for sh in range(2):
                nc.scalar.activation(ksr[:, ts(sh, 512)], ssk_ps[sh][:1, :],
                                     AF.Sqrt, bias=eps6_t[:1, :])
                nc.scalar.activation(qsr[:, ts(sh, 512)], ssq_ps[sh][:1, :],
                                     AF.Sqrt, bias=eps6_t[:1, :])
            rk_row = rowp.tile([1, S], F32, tag="rowt")
            nc.vector.reciprocal_approx_fast(out=rk_row[:], in_=ksr[:])
            rq_row = rowp.tile([1, S], F32, tag="rqrow", bufs=1)
            nc.vector.reciprocal_approx_fast(out=rq_row[:], in_=qsr[:])
            # broadcast rk over partitions; normalize k in place
            for sh in range(2):
                psb2 = pa.tile([P, 512], F32, tag="pa")
                nc.tensor.matmul(psb2[:], ones_row[:], rk_row[:, ts(sh, 512)],
                                 start=True, stop=True)
                a_k = ring.tile([P, 512], F32, tag="abc")
                nc.vector.tensor_copy(a_k[:], psb2[:])
                for dt_ in range(DSUB):
                    nc.vector.tensor_tensor(kfm[:, dt_, ts(sh, 512)],
                                            kfm[:, dt_, ts(sh, 512)], a_k[:],
                                            ALU.mult)

        # ==== chunk inverses: P_c = diag(beta) T_c^T, T = (I+N)^-1 ====
        with nc.named_scope(f"L{l}_inv"):
            # J diagonal blocks, 4 chunks packed per PSUM bank
            jall = invf.tile([P, NCH * P], F32, tag="jall")
            for half in range(2):
                jp_ps = pa.tile([P, 512], F32, tag="pa")
                nc.vector.tensor_copy(jp_ps[:], zeros_sb[:])
                for cc in range(4):
                    c = half * 4 + cc
                    for ko in range(DSUB):
                        nc.tensor.matmul(jp_ps[:, ts(cc, P)],
                                         kfm[:, ko, ts(c, P)],
                                         kfm[:, ko, ts(c, P)],
                                         start=False,
                                         stop=(cc == 3 and ko == DSUB - 1),
                                         skip_group_check=True)
                nc.vector.tensor_copy(jall[:, ts(half, 512)], jp_ps[:])
            # N (bf16) and N^T (bf16) per chunk
            nall = invb.tile([P, NCH * P], BF16, tag="nall")
            ntall = invb.tile([P, NCH * P], BF16, tag="xt0")
            for c in range(NCH):
                nsc = scr.tile([P, P], F32, tag="nscr")
                nc.vector.tensor_scalar_mul(nsc[:], jall[:, ts(c, P)],
                                            beta_tm[:, c:c + 1])
                nc.vector.tensor_tensor(nall[:, ts(c, P)], nsc[:], mask_sl[:],
                                        ALU.mult)
                bps = pb.tile([P, 256], F32, tag="pb")
                dgb = scr.tile([P, P], F32R, tag="diag")
                nc.vector.tensor_scalar_mul(dgb[:], ident_f[:],
                                            beta_tm[:, c:c + 1])
                nc.tensor.matmul(bps[:, :P], ones_sq[:], dgb[:],
                                 start=True, stop=True)
                mbx = scr.tile([P, P], F32, tag="nscr")
                nc.vector.tensor_tensor(mbx[:], bps[:, :P], mask_su[:], ALU.mult)
                nc.vector.tensor_tensor(ntall[:, ts(c, P)], mbx[:],
                                        jall[:, ts(c, P)], ALU.mult)
            # nilpotent squarings + ascending product chain:
            # T^T = (I - Y) * prod_{k>=1} (I + Y^(2^k)),  Y = N^T
            # M starts at I + Y^2 (= I + xt_1), then M += X_k^T M for k=2..6,
            # finally G = M - Y M.
            xs_cur, xt_cur = nall, ntall
            mcur = invf.tile([P, NCH * P], F32, tag="mcur")
            mb16 = invb.tile([P, NCH * P], BF16, tag="mb16")
            for kk in range(1, 7):
                xs_new = invb.tile([P, NCH * P], BF16, tag=f"xs{kk % 2}")
                for half in range(2):
                    px = pa.tile([P, 512], F32, tag="pa")
                    for cc in range(4):
                        c = half * 4 + cc
                        nc.tensor.matmul(px[:, ts(cc, P)], xt_cur[:, ts(c, P)],
                                         xs_cur[:, ts(c, P)], start=True,
                                         stop=True, skip_group_check=True)
                    nc.vector.tensor_copy(xs_new[:, ts(half, 512)], px[:])
                if kk < 6:
                    xt_new = invb.tile([P, NCH * P], BF16, tag=f"xt{kk % 2}")
                    for half in range(2):
                        px = pa.tile([P, 512], F32, tag="pa")
                        for cc in range(4):
                            c = half * 4 + cc
                            nc.tensor.matmul(px[:, ts(cc, P)], xs_cur[:, ts(c, P)],
                                             xt_cur[:, ts(c, P)], start=True,
                                             stop=True, skip_group_check=True)
                        nc.vector.tensor_copy(xt_new[:, ts(half, 512)], px[:])
                if kk == 1:
                    # M = I + Y^2 = I + xt_new(level 1)
                    for c in range(NCH):
                        nc.vector.tensor_tensor(mcur[:, ts(c, P)], ident_f[:],
                                                xt_new[:, ts(c, P)], ALU.add)
                    nc.vector.tensor_copy(mb16[:, :512], mcur[:, :512])
                    nc.vector.tensor_copy(mb16[:, 512:], mcur[:, 512:])
                else:
                    # M += X_kk^T M  (xs_new holds X_kk = Y^(2^kk) transposed
                    # chain: lhsT = xs_new gives xs_new^T @ M)
                    for half in range(2):
                        pm = pa.tile([P, 512], F32, tag="pa")
                        for cc in range(4):
                            c = half * 4 + cc
                            nc.tensor.matmul(pm[:, ts(cc, P)], xs_new[:, ts(c, P)],
                                             mb16[:, ts(c, P)], start=True,
                                             stop=True, skip_group_check=True)
                        nc.vector.tensor_tensor(mcur[:, ts(half, 512)],
                                                mcur[:, ts(half, 512)], pm[:],
                                                ALU.add)
                        nc.vector.tensor_copy(mb16[:, ts(half, 512)],
                                              mcur[:, ts(half, 512)])
                xs_cur = xs_new
                if kk < 6:
                    xt_cur = xt_new
            # G = M - Y M ; P = beta * G ; Pneg = -beta * G
            pall = invb.tile([P, NCH * P], F32R, tag="pall")
            pneg = invb.tile([P, NCH * P], BF16, tag="pneg")
            for half in range(2):
                pm = pa.tile([P, 512], F32, tag="pa")
                for cc in range(4):
                    c = half * 4 + cc
                    nc.tensor.matmul(pm[:, ts(cc, P)], nall[:, ts(c, P)],
                                     mb16[:, ts(c, P)], start=True, stop=True,
                                     skip_group_check=True)
                for cc in range(4):
                    c = half * 4 + cc
                    gtc = scr.tile([P, P], F32, tag="nscr")
                    nc.vector.tensor_tensor(gtc[:], mcur[:, ts(c, P)],
                                            pm[:, ts(cc, P)], ALU.subtract)
                    nc.vector.tensor_scalar_mul(pall[:, ts(c, P)], gtc[:],
                                                beta_tm[:, c:c + 1])
                    nc.vector.tensor_scalar(pneg[:, ts(c, P)], gtc[:],
                                            beta_tm[:, c:c + 1], -1.0,
                                            ALU.mult, ALU.mult)

        # ==== scan ====
        with nc.named_scope(f"L{l}_scan"):
            sso_row = rowp.tile([1, S], F32, tag="ssor", bufs=1,
                                name=f"sso{l}")
            for cp in range(4):
                c0, c1 = 2 * cp, 2 * cp + 1
                for c in (c0, c1):
                    # J pair tiles for j < c
                    jsbs = []
                    for jp in range((c + 1) // 2):
                        jps = pb.tile([P, 256], F32, tag="pb")
                        for ko in range(DSUB):
                            nc.tensor.matmul(jps[:], kfm[:, ko, ts(c, P)],
                                             kfm[:, ko, ts(jp, 256)],
                                             start=(ko == 0),
                                             stop=(ko == DSUB - 1))
                        jsb = jpool.tile([P, 256], BF16, tag="jsb")
                        nc.vector.tensor_copy(jsb[:], jps[:])
                        jsbs.append(jsb)
                    # -G tiles packed 4 per bank
                    gnegs = []
                    for gi in range(0, c, 4):
                        nb = min(4, c - gi)
                        gp = pa.tile([P, 512], F32, tag="pa")
                        for jj in range(nb):
                            j = gi + jj
                            nc.tensor.matmul(gp[:, ts(jj, P)],
                                             jsbs[j // 2][:, ts(j % 2, P)],
                                             pneg[:, ts(c, P)], start=True,
                                             stop=True, skip_group_check=True)
                        gsb = gpool.tile([P, 512], F32R, tag="gneg")
                        nc.vector.tensor_copy(gsb[:, :nb * P], gp[:, :nb * P])
                        gnegs.append((gsb, nb))
                    # U_c = P_c^T V_c - sum_j G_cj U_j   (in place over v)
                    for half in range(2):
                        psu = pa.tile([P, 512], F32, tag="pa")
                        nc.tensor.matmul(psu[:], pall[:, ts(c, P)],
                                         u_tm[:, c, ts(half, 512)],
                                         start=True, stop=(c == 0))
                        jx = 0
                        for gsb, nb in gnegs:
                            for jj in range(nb):
                                nc.tensor.matmul(psu[:], gsb[:, ts(jj, P)],
                                                 u_tm[:, jx, ts(half, 512)],
                                                 start=False,
                                                 stop=(jx == c - 1))
                                jx += 1
                        nc.vector.tensor_copy(u_tm[:, c, ts(half, 512)], psu[:])
                # H^T pair tiles for this cp
                hps = []
                for j in range(c1 + 1):
                    php = pb.tile([P, 256], F32, tag="pb")
                    for ko in range(DSUB):
                        nc.tensor.matmul(php[:], kfm[:, ko, ts(j, P)],
                                         qfm[:, ko, ts(cp, 256)],
                                         start=(ko == 0), stop=(ko == DSUB - 1))
                    hp = hppool.tile([P, 256], F32R, tag="hp")
                    if j == c0:
                        nc.vector.tensor_tensor(hp[:, :P], php[:, :P], mask_ui[:],
                                                ALU.mult)
                        nc.vector.tensor_copy(hp[:, P:], php[:, P:])
                    elif j == c1:
                        nc.vector.tensor_tensor(hp[:, P:], php[:, P:], mask_ui[:],
                                                ALU.mult)
                    else:
                        nc.vector.tensor_copy(hp[:], php[:])
                    hps.append(hp)
                # O feature-major, written into qfm (over q), sumsq columns
                for et in range(DSUB):
                    pso = pb.tile([P, 256], F32, tag="pb")
                    for j in range(c1 + 1):
                        if j == c1:
                            nc.tensor.matmul(pso[:, P:], u_tm[:, j, ts(et, P)],
                                             hps[j][:, P:], start=False, stop=True)
                        else:
                            nc.tensor.matmul(pso[:], u_tm[:, j, ts(et, P)],
                                             hps[j][:], start=(j == 0), stop=False)
                    nc.vector.tensor_copy(qfm[:, et, ts(cp, 256)], pso[:])
                sop = pb.tile([P, 256], F32, tag="pb", name=f"sop{l}_{cp}")
                for et in range(DSUB):
                    sqo = scr.tile([P, 512], F32R, tag="scr")
                    nc.vector.tensor_tensor(sqo[:, :256], qfm[:, et, ts(cp, 256)],
                                            qfm[:, et, ts(cp, 256)], ALU.mult)
                    nc.tensor.matmul(sop[:1, :256], ones_r[:], sqo[:, :256],
                                     start=(et == 0), stop=(et == DSUB - 1))
                nc.vector.tensor_copy(sso_row[:, ts(cp, 256)], sop[:1, :256])

        # ==== x_next = rowa * (on @ Wo'), Wo' = rms_w-scaled Wo ====
        with nc.named_scope(f"L{l}_oproj"):
            for dt_ in range(DSUB):
                wt = wko.tile([P, DSUB, P], BF16, tag="wo")
                nc.sync.dma_start(wt[:], wo_d[l, :, :, ts(dt_, P)])
                for sh in range(2):
                    px = pa.tile([P, 512], F32, tag="pa")
                    for ko in range(DSUB):
                        nc.tensor.matmul(px[:], wt[:, ko, :],
                                         qfm[:, ko, ts(sh, 512)],
                                         start=(ko == 0), stop=(ko == DSUB - 1))
                    nc.vector.tensor_copy(xfm[:, dt_, ts(sh, 512)], px[:])
            # rowa = rq / sqrt(rq^2 * sso / D + eps_rms)
            rq2 = rowp.tile([1, S], F32, tag="rowt")
            nc.vector.tensor_tensor(rq2[:], rq_row[:], rq_row[:], ALU.mult)
            nc.vector.tensor_tensor(rq2[:], rq2[:], sso_row[:], ALU.mult)
            ra_s = rowp.tile([1, S], F32, tag="rowt")
            nc.scalar.activation(ra_s[:], rq2[:], AF.Sqrt, scale=invd_t[:1, :],
                                 bias=eps5_t[:1, :])
            ra_row = rowp.tile([1, S], F32, tag="rowt")
            nc.vector.reciprocal_approx_fast(out=ra_row[:], in_=ra_s[:])
            nc.vector.tensor_tensor(ra_row[:], ra_row[:], rq_row[:], ALU.mult)
            for sh in range(2):
                psb2 = pa.tile([P, 512], F32, tag="pa")
                nc.tensor.matmul(psb2[:], ones_row[:], ra_row[:, ts(sh, 512)],
                                 start=True, stop=True)
                a_bc = ring.tile([P, 512], F32, tag="abc")
                nc.vector.tensor_copy(a_bc[:], psb2[:])
                for dt_ in range(DSUB):
                    nc.vector.tensor_tensor(xfm[:, dt_, ts(sh, 512)],
                                            xfm[:, dt_, ts(sh, 512)], a_bc[:],
                                            ALU.mult)

    # ==== final layernorm (g/b folded into head weights/bias on host) ====
    with nc.named_scope("ln"):
        sum_ps = [pa.tile([P, 512], F32, tag="pa", name=f"lnsum{i}")
                  for i in range(2)]
        ssq_ps = [pa.tile([P, 512], F32, tag="pa", name=f"lnssq{i}")
                  for i in range(2)]
        for dt_ in range(DSUB):
            for sh in range(2):
                nc.tensor.matmul(sum_ps[sh][:1, :], ones_r[:],
                                 xfm[:, dt_, ts(sh, 512)],
                                 start=(dt_ == 0), stop=(dt_ == DSUB - 1))
                sqx = scr.tile([P, 512], F32R, tag="scr")
                nc.vector.tensor_tensor(sqx[:], xfm[:, dt_, ts(sh, 512)],
                                        xfm[:, dt_, ts(sh, 512)], ALU.mult)
                nc.tensor.matmul(ssq_ps[sh][:1, :], ones_r[:], sqx[:],
                                 start=(dt_ == 0), stop=(dt_ == DSUB - 1))
        mu = rowp.tile([1, S], F32, tag="lnmu", bufs=1)
        m2 = rowp.tile([1, S], F32, tag="rowt")
        for sh in range(2):
            nc.vector.tensor_scalar_mul(mu[:, ts(sh, 512)], sum_ps[sh][:1, :],
                                        1.0 / D)
            nc.vector.tensor_scalar_mul(m2[:, ts(sh, 512)], ssq_ps[sh][:1, :],
                                        1.0 / D)
        mu2 = rowp.tile([1, S], F32, tag="rowt")
        nc.vector.tensor_tensor(mu2[:], mu[:], mu[:], ALU.mult)
        nc.vector.tensor_tensor(m2[:], m2[:], mu2[:], ALU.subtract)
        sd = rowp.tile([1, S], F32, tag="rowt")
        nc.scalar.activation(sd[:], m2[:], AF.Sqrt, bias=eps5_t[:1, :])
        rr = rowp.tile([1, S], F32, tag="rowt")
        nc.vector.reciprocal_approx_fast(out=rr[:], in_=sd[:])
        nb_ = rowp.tile([1, S], F32, tag="rowt")
        nc.vector.tensor_tensor(nb_[:], mu[:], rr[:], ALU.mult)
        nc.vector.tensor_scalar_mul(nb_[:], nb_[:], -1.0)
        for sh in range(2):
            psb2 = pa.tile([P, 512], F32, tag="pa")
            nc.tensor.matmul(psb2[:], ones_row[:], rr[:, ts(sh, 512)],
                             start=True, stop=True)
            a_bc = ring.tile([P, 512], F32, tag="abc")
            nc.vector.tensor_copy(a_bc[:], psb2[:])
            psb3 = pa.tile([P, 512], F32, tag="pa")
            nc.tensor.matmul(psb3[:], ones_row[:], nb_[:, ts(sh, 512)],
                             start=True, stop=True)
            b_bc = ring.tile([P, 512], F32, tag="abc")
            nc.vector.tensor_copy(b_bc[:], psb3[:])
            for dt_ in range(DSUB):
                nc.vector.tensor_tensor(xfm[:, dt_, ts(sh, 512)],
                                        xfm[:, dt_, ts(sh, 512)], a_bc[:],
                                        ALU.mult)
                nc.vector.tensor_tensor(xfm[:, dt_, ts(sh, 512)],
                                        xfm[:, dt_, ts(sh, 512)], b_bc[:],
                                        ALU.add)

    # ==== vocab-shard head: logits_t[vt*128+vv, s] (bf16 out + bias) ====
    with nc.named_scope("head"):
        for vt in range(VTS):
            hwts = []
            for kw in range(2):
                hwt = hwp.tile([P, 4, P], F32R, tag="hw", name=f"hw{vt}_{kw}")
                nc.sync.dma_start(hwt[:], hw_d[vt, :, ts(kw, 4), :])
                hwts.append(hwt)
            for sh in range(2):
                ps = pa.tile([P, 512], F32, tag="pa")
                for ko in range(DSUB):
                    nc.tensor.matmul(ps[:], hwts[ko // 4][:, ko % 4, :],
                                     xfm[:, ko, ts(sh, 512)],
                                     start=(ko == 0), stop=(ko == DSUB - 1))
                ot = outp.tile([P, 512], BF16, tag="out")
                nc.vector.tensor_scalar_add(ot[:], ps[:], hb_sb[:, vt:vt + 1])
                nc.sync.dma_start(out_d[ts(vt, P), ts(sh, 512)], ot[:])

    ctx.close()


def _round_f32r(x):
    m, e = np.frexp(x.astype(np.float64))
    return np.ldexp(np.round(m * 4096.0) / 4096.0, e).astype(np.float32)


_CACHE = {}


def _get_program():
    if "nc" not in _CACHE:
        _CACHE["nc"] = build_program()
    return _CACHE["nc"]


def make_in_maps(tokens, emb, Wq, Wk, Wv, Wb, Wo, rms_w, ln_g, ln_b, head_w):
    def arrange_w(w):  # [D, N] -> [128, DSUB, N] with (p, ko) striping of D
        return np.ascontiguousarray(
            _round_f32r(w).reshape(DSUB, P, -1).transpose(1, 0, 2))

    wq_h = np.stack([arrange_w(Wq[l]) for l in range(L)])
    wk_h = np.stack([arrange_w(Wk[l]) for l in range(L)])
    wv_h = np.stack([arrange_w(Wv[l]) for l in range(L)])
    wb_h = np.stack([arrange_w(np.repeat(Wb[l], 2, axis=1)) for l in range(L)])
    wo_h = np.stack([
        np.ascontiguousarray((rms_w[l][:, None] * Wo[l]).astype(
            ml_dtypes.bfloat16).reshape(DSUB, P, -1).transpose(1, 0, 2))
        for l in range(L)])
    emb_h = _round_f32r(emb)
    hw_eff = ln_g[:, None].astype(np.float64) * head_w.astype(np.float64)
    bias_full = (ln_b.astype(np.float64) @ head_w.astype(np.float64))

    in_maps = []
    for core in range(8):
        b, vs = core // 4, core % 4
        hw_pad = np.zeros((D, VSP), np.float32)
        hw_pad[:, :VS] = _round_f32r(hw_eff[:, ts(vs, VS)])
        hw_h = np.ascontiguousarray(
            hw_pad.reshape(DSUB, P, VTS, P).transpose(2, 1, 0, 3))
        hb_pad = np.zeros((VSP,), np.float32)
        hb_pad[:VS] = bias_full[ts(vs, VS)].astype(np.float32)
        hb_h = np.ascontiguousarray(hb_pad.reshape(VTS, P).T)
        tok_h = np.ascontiguousarray(
            tokens[b].astype(np.int32).reshape(NCH, P).T)
        in_maps.append({
            "tokens": tok_h, "emb": emb_h,
            "wq": wq_h, "wk": wk_h, "wv": wv_h, "wb": wb_h, "wo": wo_h,
            "hw": hw_h, "hb": hb_h,
        })
    return in_maps


def assemble_output(results):
    out = np.empty((2, S, V), np.float32)
    for core in range(8):
        b, vs = core // 4, core % 4
        lt = results[core]["logits_t"]          # [VSP, S] bf16
        out[b, :, ts(vs, VS)] = np.ascontiguousarray(
            lt[:VS].astype(np.float32)).T
    return out


def kernel(tokens, emb, Wq, Wk, Wv, Wb, Wo, rms_w, ln_g, ln_b, head_w):
    tokens = np.asarray(tokens)
    args = [np.asarray(a, np.float32) for a in
            (emb, Wq, Wk, Wv, Wb, Wo, rms_w, ln_g, ln_b, head_w)]
    nc = _get_program()
    in_maps = make_in_maps(tokens, *args)
    res = run_bass_kernel_spmd(nc, in_maps, core_ids=list(range(8)),
                               trace=bool(_CACHE.get("trace")))
    _CACHE["last_result"] = res
    return assemble_output(res.results)
